# revision 69
# baseline (speedup 1.0000x reference)
"""Bahdanau-attention scoring kernel for 8 TRN2 NeuronCores (fp8 DoubleRow).

Reference computation (S=2048, B=32, H=1024):
    cat    = concat([broadcast(hidden), enc], axis=2)          # [S,B,2H]
    alphas = tanh(einsum('sbk,hk->sbh', cat, W_attn) + b_attn) # [S,B,H]
    scores = einsum('sbh,h->sb', alphas, v)                    # [S,B]
    out    = softmax(scores.T, axis=1)[:, None, :]             # [B,1,S]

Split W_attn = [W1 | W2]: z[s,b,:] = W2 @ enc[s,b,:] + hp[b,:] with
hp[b,:] = W1 @ hidden[b,:] + b_attn computed once per batch.

Layout: h_out on partitions, s on the free dim.  The dominant matmul
(S*B*H*H MACs) runs in fp8 e4m3 DoubleRow mode - one instruction
contracts TWO 128-deep k-tiles, doubling PE throughput vs bf16.  W2 is
pre-scaled by 32 on the host so its entries clear e4m3's subnormal
floor; the 1/32 rides the ACT tanh's free scale operand.  hp[b] lands
as a per-partition bias AP on the same tanh, and the v-contraction
(over h = partitions) is a tiny K=128 PE matmul accumulating [1,512]
score chunks in PSUM - the Vector engine does only the softmax tail.

hp path: W1/hidden in fp8 e3m4 (4-bit mantissa, *128 pre-scale), with
w1 blocks stationary so hp comes out already h-on-partitions [128, 4].

Sharding: data-parallel over batch.  Core c handles batches 4c..4c+3.
"""

import sys

for _p in ("/opt/trn_rl_repo", "/root/.axon_site/_ro/trn_rl_repo"):
    if _p not in sys.path:
        sys.path.insert(0, _p)

import numpy as np
import ml_dtypes

import concourse.bass as bass  # noqa: F401  (bass must import before tile)
import concourse.mybir as mybir
import concourse.tile as tile
from concourse import bacc
from concourse.bass_utils import run_bass_kernel_spmd

S, B, H = 2048, 32, 1024
NCORES = 8
BL = B // NCORES          # batches per core (4)
P = 128                   # SBUF partitions
HT = H // P               # k-tiles over h_in (8)
SC = 512                  # s-chunk per enc DMA / psum tile
NSC = S // SC             # s chunks per batch row (4)
NHC = H // P              # h_out blocks of 128 (8)
KP = HT // 2              # DoubleRow k-pairs per z group (4)

W2SCALE = 32.0            # pre-scale so W2 clears e4m3's subnormal floor
W1SCALE = 128.0           # same for W1 in e3m4 (max |W1*128| ~ 14 < 15.5)

F8E4 = mybir.dt.float8e4
F8E3 = mybir.dt.float8e3
BF16 = mybir.dt.bfloat16
F32 = mybir.dt.float32
AFT = mybir.ActivationFunctionType
DR = mybir.MatmulPerfMode.DoubleRow
MUL = mybir.AluOpType.mult
ADD = mybir.AluOpType.add

_nc_cache = None


def build():
    nc = bacc.Bacc()
    enc = nc.declare_dram_parameter("enc", [BL, H, S], F8E4, isOutput=False)
    w2t = nc.declare_dram_parameter("w2t", [H, H], F8E4, isOutput=False)
    w1t = nc.declare_dram_parameter("w1t", [H, H], F8E3, isOutput=False)
    hid = nc.declare_dram_parameter("hid", [P, HT, BL], F8E3, isOutput=False)
    ba = nc.declare_dram_parameter("ba", [P, HT], BF16, isOutput=False)
    vp = nc.declare_dram_parameter("v", [P, HT], BF16, isOutput=False)
    out = nc.declare_dram_parameter("out", [BL, S], F32, isOutput=True)

    with tile.TileContext(nc) as tc:
        with (
            tc.tile_pool(name="const", bufs=1) as cpool,
            tc.tile_pool(name="encp", bufs=4) as encp,
            tc.tile_pool(name="alp", bufs=28) as alp,
            tc.tile_pool(name="sxp", bufs=2) as sxp,
            tc.tile_pool(name="osp", bufs=3) as osp,
            tc.tile_pool(name="smallp", bufs=6) as smallp,
            tc.tile_pool(name="zps", bufs=5, space="PSUM") as zps,
            tc.tile_pool(name="scps", bufs=1, space="PSUM") as scps,
            tc.tile_pool(name="hpps", bufs=1, space="PSUM") as hpps,
        ):
            # --- PE pstate warm-up ---
            # The PE ramps 0.65 -> 1.2 -> 2.4 GHz over ~3us of continuous
            # work; the first real z matmuls otherwise run 2-3x slow while
            # ramping.  Dummy matmuls on memset data need no DMA, so they
            # start right after the framework preamble (~6.5us) and finish
            # before the first enc/W2 chunks land (~11.5us).
            wup_w = cpool.tile([P, P], BF16)
            nc.vector.memset(wup_w[:], 0.0)
            wup_m = cpool.tile([P, SC], BF16)
            nc.vector.memset(wup_m[:], 0.0)
            for i in range(12):
                wup_ps = zps.tile([P, SC], F32, tag="z", name=f"wup{i}")
                nc.tensor.matmul(wup_ps[:], wup_w[:], wup_m[:],
                                 start=True, stop=True)

            # --- resident constants ---
            # scalar (ACT) hwdge queue: hid is pre-shaped [P, HT, BL] on the
            # host so its DMA is contiguous (the old "(t p) b -> p t b"
            # rearrange descriptor gathered 8-byte segments and stalled the
            # whole queue ~3us); W1 rides as contiguous kt-row DMAs.  ba
            # slots after the first two W1 rows - the hp kt-blocks need W1
            # early, the b_attn add (DVE finish) only at hp-group end.
            hid_sb = cpool.tile([P, HT, BL], F8E3)
            nc.scalar.dma_start(hid_sb[:], hid[:])
            w1_sb = cpool.tile([P, HT, H], F8E3)
            for kt in range(2):
                nc.scalar.dma_start(w1_sb[:, kt, :], w1t[kt * P:(kt + 1) * P, :])
            ba_sb = cpool.tile([P, HT], BF16)
            nc.scalar.dma_start(ba_sb[:], ba[:])
            # W1 rows split scalar/gpsimd: ordered-DMA pacing is ~1.2us per
            # descriptor per queue, so one queue alone lands row 7 too late
            for kt in range(2, HT - 2):
                nc.scalar.dma_start(w1_sb[:, kt, :], w1t[kt * P:(kt + 1) * P, :])
            v_sb = cpool.tile([P, HT], BF16)
            nc.scalar.dma_start(v_sb[:], vp[:])
            # W2 on sync, enc chunk 0 on gpsimd (parallel DMA engines), both
            # as kt-PAIR tiles so DR matmul j only waits its own pair - the
            # first z matmul issues ~2us in instead of waiting the full 1.5MB
            w2p = [cpool.tile([P, 2, H], F8E4, tag=f"w2p{j}", name=f"w2p{j}")
                   for j in range(KP)]
            et0p = [cpool.tile([P, 2, SC], F8E4, tag=f"et0p{j}", name=f"et0p{j}")
                    for j in range(KP)]
            for j in range(KP):
                for t in range(2):
                    kt = 2 * j + t
                    nc.sync.dma_start(w2p[j][:, t, :], w2t[kt * P:(kt + 1) * P, :])
                    nc.gpsimd.dma_start(
                        et0p[j][:, t, :], enc[0, kt * P:(kt + 1) * P, 0:SC])
            for kt in range(HT - 2, HT):
                nc.gpsimd.dma_start(w1_sb[:, kt, :], w1t[kt * P:(kt + 1) * P, :])
            hp_sb = cpool.tile([P, HT, BL], F32)

            # hp[:, hc, b] = (W1 @ hidden[b] + b_attn)[hc*128:(hc+1)*128].
            # All 8 h_out blocks accumulate in ONE single-bank psum tile as
            # ONE group (first mm's start=True marks the whole bank pending-
            # zero), emitted kt-major so each 8-mm block only needs w1 row kt
            # - the hp path trickles in behind the W1 row DMAs with no psum
            # pool coupling to the z pipeline.
            hp_ps = hpps.tile([P, NHC, BL], F32)

            def emit_hp_kt(kt):
                for hc in range(NHC):
                    nc.tensor.matmul(
                        hp_ps[:, hc, :], w1_sb[:, kt, hc * P:(hc + 1) * P],
                        hid_sb[:, kt, :],
                        start=(kt == 0 and hc == 0),
                        stop=(kt == HT - 1 and hc == NHC - 1),
                        skip_group_check=True)

            def emit_hp_finish():
                # rescale + add b_attn on the (idle) vector engine
                for hc in range(NHC):
                    nc.vector.scalar_tensor_tensor(
                        hp_sb[:, hc, :], hp_ps[:, hc, :], 1.0 / W1SCALE,
                        ba_sb[:, hc:hc + 1].broadcast_to((P, BL)),
                        op0=MUL, op1=ADD)

            # v-dots + softmax pieces for one chunk, deferred two chunks
            # behind the z matmuls so every al is long since produced and
            # the scheduler naturally clusters runs of ~4 v-dots (fewer
            # DR<->normal mode switches; ~235ns per switch pair).  Measured
            # alternatives - per-chunk scores tiles, dual-region psum,
            # tc.high_priority, 3-chunk deferral - all cluster WORSE.
            def flush_chunk(pend):
                b, sc, als, scores, exs, parts = pend
                h = sc % 2
                for hc, al in enumerate(als):
                    nc.tensor.matmul(
                        scores[:, h * SC:(h + 1) * SC],
                        v_sb[:, hc:hc + 1], al[:],
                        start=(hc == 0), stop=(hc == NHC - 1),
                        skip_group_check=True)
                if h == 1:
                    # no max-sub: |scores| <= sum|v| ~ 26 fits exp in f32
                    nc.scalar.activation(
                        exs[:, (sc - 1) * SC:(sc + 1) * SC], scores[:],
                        AFT.Exp, accum_out=parts[:, sc // 2:sc // 2 + 1])
                if sc == NSC - 1:
                    tot = smallp.tile([1, 1], F32, tag="tot")
                    nc.vector.tensor_add(tot[:], parts[:, 0:1], parts[:, 1:2])
                    rec = smallp.tile([1, 1], F32, tag="rec")
                    nc.vector.reciprocal(rec[:], tot[:])
                    osb = osp.tile([1, S], F32, tag="osb")
                    nc.vector.tensor_scalar_mul(osb[:], exs[:], rec[:, 0:1])
                    nc.sync.dma_start(out[b:b + 1, :], osb[:])

            # --- main loop ---
            first = True
            ci = 0
            pending = []
            for b in range(BL):
                exs = sxp.tile([1, S], F32, tag="ex")
                parts = smallp.tile([1, 2], F32, tag="part")
                for sc in range(NSC):
                    if sc % 2 == 0:
                        scores = scps.tile([1, 2 * SC], F32, tag="sc")
                    if not first:
                        et = encp.tile([P, HT, SC], F8E4, tag="enc")
                        q = nc.gpsimd if ci % 2 == 0 else nc.sync
                        q.dma_start(
                            et[:],
                            enc[b, :, sc * SC:(sc + 1) * SC].rearrange(
                                "(t p) s -> p t s", p=P))
                    ci += 1
                    # flush two chunks behind: by then every tanh of that
                    # chunk is long done, so all 8 v-dots are ready at once
                    # and the scheduler clusters them (one DR<->normal mode
                    # switch per cluster instead of eight)
                    if len(pending) == 2:
                        flush_chunk(pending.pop(0))

                    def z_group(hc, et_):
                        z = zps.tile([P, SC], F32, tag="z", name=f"z{hc}")
                        for j in range(KP):
                            nc.tensor.matmul(
                                z[:],
                                w2p[j][:, :, hc * P:(hc + 1) * P],
                                et0p[j][:] if et_ is None
                                else et_[:, 2 * j:2 * j + 2, :],
                                start=(j == 0), stop=(j == KP - 1),
                                perf_mode=DR)
                        return z

                    def tanh(hc, z):
                        al = alp.tile([P, SC], BF16, tag="al")
                        nc.scalar.activation(
                            al[:], z[:], AFT.Tanh,
                            bias=hp_sb[:, hc, b:b + 1], scale=1.0 / W2SCALE)
                        return al

                    als = []
                    if first:
                        # hp kt-blocks pair with the first 4 z groups; all hp
                        # emission (incl. the DVE finish that writes hp_sb)
                        # precedes the first tanh, and precedes z4-z7 so the
                        # tanh->zpool chain never waits on later PE work
                        zs = []
                        for hc in range(4):
                            zs.append(z_group(hc, None))
                            emit_hp_kt(hc)
                        for kt in range(4, HT):
                            emit_hp_kt(kt)
                        emit_hp_finish()
                        for hc in range(4):
                            als.append(tanh(hc, zs[hc]))
                        for hc in range(4, NHC):
                            als.append(tanh(hc, z_group(hc, None)))
                    else:
                        for hc in range(NHC):
                            als.append(tanh(hc, z_group(hc, et)))
                    first = False
                    pending.append((b, sc, als, scores, exs, parts))
            for p in pending:
                flush_chunk(p)
    nc.compile()
    return nc


def _get_nc():
    global _nc_cache
    if _nc_cache is None:
        _nc_cache = build()
    return _nc_cache


def _prep_inputs(hidden, encoder_outputs, W_attn, b_attn, v):
    e4 = ml_dtypes.float8_e4m3
    e3 = ml_dtypes.float8_e3m4
    bf = ml_dtypes.bfloat16
    hidden = np.asarray(hidden, dtype=np.float32)
    encoder_outputs = np.asarray(encoder_outputs, dtype=np.float32)
    W_attn = np.asarray(W_attn, dtype=np.float32)
    b_attn = np.asarray(b_attn, dtype=np.float32)
    v = np.asarray(v, dtype=np.float32)

    W1 = W_attn[:, :H]
    W2 = W_attn[:, H:]
    w2t = np.clip(W2.T * W2SCALE, -240.0, 240.0).astype(e4)     # [H kin, H hout]
    w1t = np.clip(W1.T * W1SCALE, -15.5, 15.5).astype(e3)
    # [P, HT, B]: pre-shaped so the on-device hid DMA is fully contiguous
    hid_t = np.clip(hidden[0].T, -15.5, 15.5).astype(e3).reshape(
        HT, P, B).transpose(1, 0, 2)
    ba = np.ascontiguousarray(b_attn.reshape(HT, P).T).astype(bf)  # [P, HT]
    vpt = np.ascontiguousarray(v.reshape(HT, P).T).astype(bf)   # [P, HT]
    enc_t = encoder_outputs.transpose(1, 2, 0).astype(e4)       # [B, H, S]

    in_maps = []
    for c in range(NCORES):
        bsl = slice(c * BL, (c + 1) * BL)
        in_maps.append({
            "enc": np.ascontiguousarray(enc_t[bsl]),
            "w2t": w2t,
            "w1t": w1t,
            "hid": np.ascontiguousarray(hid_t[:, :, bsl]),
            "ba": ba,
            "v": vpt,
        })
    return in_maps


def kernel(hidden, encoder_outputs, W_attn, b_attn, v, _trace=False):
    in_maps = _prep_inputs(hidden, encoder_outputs, W_attn, b_attn, v)
    nc = _get_nc()
    res = run_bass_kernel_spmd(
        nc, in_maps, core_ids=list(range(NCORES)), trace=_trace,
    )
    parts = [res.results[c]["out"] for c in range(NCORES)]      # each [BL, S]
    full = np.concatenate(parts, axis=0)                        # [B, S]
    out = full[:, None, :].astype(np.float32)                   # [B, 1, S]
    if _trace:
        return out, res
    return out
